# revision 7
# baseline (speedup 1.0000x reference)
"""Causal multi-head self-attention (B=2, T=2048, D=1024, H=16) on 8 TRN2
NeuronCores.

Sharding (Megatron-style, hardcoded): core = 4*b + g where b in {0,1} is the
batch and g in {0..3} a group of 4 heads. Each core computes Q/K/V projections
for its head group from x[b], fused causal attention for those 4 heads, and a
partial output projection against its 256-column slice of Wo. The host sums
the 4 partial outputs per batch (the all-reduce after out_proj).

v2 design (all matmul operands bf16, fp32 PSUM accumulation):
 - Scores per (kti, head-pair): a 4-way tiled quad of M=64 matmuls covering
   all 16 PE subarray quadrants (rows split by head, cols by k-half) -> both
   heads' score tiles land in one 512-cycle stream.
 - PV per (kti, head-pair): col-tiled M=64 pair (head0 -> PSUM rows 0:64,
   head1 -> 64:128) accumulating over kti.
 - Softmax denominators: 4-way col-tiled quad of M=1 ones-matmuls (one per
   head) accumulating into a single PSUM bank at partitions {0,32,64,96}.
 - Attention is software-pipelined: scores+exp for kti overlap PV+sums for
   kti-1, so PE never waits on the scalar engine's exp backlog.
 - Causal diagonal tiles: exp is sliced to the valid column range; the
   128-wide triangle is masked by multiplying with a precomputed lower-tri
   bf16 mask on GpSimd; dead columns live in dedicated per-(offset, head-pair)
   pt tiles whose masked region is zeroed once at startup.
 - Normalize: gather the 4 denominator rows to [16,128], one multi-pass DVE
   reciprocal, scatter back, gpsimd partition_broadcast, then a DVE multiply
   that reads the PV accumulator directly from PSUM and writes bf16 `at`.
 - Input DMAs are chunked (weights and x in halves) so the first projection
   matmul can start early.
"""

import numpy as np
import ml_dtypes

import concourse.bass as bass
import concourse.tile as tile
from concourse import bacc, mybir
from concourse.bass_utils import run_bass_kernel_spmd

B, T, D, H, DH = 2, 2048, 1024, 16, 64
HPC = 4  # heads per core
GC = 256  # projection columns per core (HPC * DH)
N_CORES = 8
F32 = mybir.dt.float32
BF16 = mybir.dt.bfloat16
EXP = mybir.ActivationFunctionType.Exp

_CACHE = {}


def _build():
    nc = bacc.Bacc(
        "TRN2", target_bir_lowering=False, debug=False, num_devices=N_CORES
    )
    # Pre-swizzled bf16 inputs (host does transposes + cast):
    #   xs[p, tc, dt, t] = x[b, tc*512+t, dt*128+p]
    #   wq/wk/wv[p, dt, c] = W[g*256+c, dt*128+p]
    #   wo[p, ct, n] = Wo[n, g*256 + ct*128 + p]
    xs = nc.dram_tensor("xs", [128, 4, 8, 512], BF16, kind="ExternalInput").ap()
    wqs = nc.dram_tensor("wqs", [128, 8, GC], BF16, kind="ExternalInput").ap()
    wks = nc.dram_tensor("wks", [128, 8, GC], BF16, kind="ExternalInput").ap()
    wvs = nc.dram_tensor("wvs", [128, 8, GC], BF16, kind="ExternalInput").ap()
    wos = nc.dram_tensor("wos", [128, 2, D], BF16, kind="ExternalInput").ap()
    out = nc.dram_tensor("out", [T, D], F32, kind="ExternalOutput").ap()

    with tile.TileContext(nc) as tc:
        with (
            tc.tile_pool(name="persist", bufs=1) as persist,
            tc.tile_pool(name="xtp", bufs=2) as xtp,
            tc.tile_pool(name="ptp", bufs=4) as ptp,
            tc.tile_pool(name="normp", bufs=2) as normp,
            tc.tile_pool(name="outp", bufs=2) as outp,
            # PSUM (8 banks): opv0+opv1+sums = 3, st (2 banks) x2 = 4, aux = 1
            tc.tile_pool(name="pvp", bufs=1, space="PSUM") as pvp,
            tc.tile_pool(name="stp", bufs=2, space="PSUM") as stp,
            tc.tile_pool(name="auxp", bufs=1, space="PSUM") as auxp,
        ):
            wq = persist.tile([128, 8, GC], BF16, tag="wq")
            wk = persist.tile([128, 8, GC], BF16, tag="wk")
            wv = persist.tile([128, 8, GC], BF16, tag="wv")
            wo = persist.tile([128, 2, D], BF16, tag="wo")
            qt = persist.tile([128, 2, T], BF16, tag="qt")
            kt = persist.tile([128, 2, T], BF16, tag="kt")
            vp = persist.tile([128, 16, HPC, DH], BF16, tag="vp")
            at = persist.tile([128, 2, T], BF16, tag="at")
            onesb = persist.tile([128, 4], BF16, tag="onesb")
            trimask = persist.tile([128, 2, 128], BF16, tag="trimask")
            # dedicated diagonal pt tiles per (offset j, head pair hp);
            # cols [0, 128j) are zeroed once and never rewritten
            ptdiag = [
                [persist.tile([128, 2, 512], BF16, tag=f"ptd{j}_{hp}",
                              name=f"ptd{j}_{hp}")
                 for hp in range(2)]
                for j in range(4)
            ]

            # Input DMA order: first what phase 1 needs first.
            nc.sync.dma_start(wq[:, 0:4, :], wqs[:, 0:4, :])
            x0 = xtp.tile([128, 8, 512], BF16, tag="xt")
            nc.sync.dma_start(x0[:, 0:4, :], xs[:, 0, 0:4, :])
            nc.sync.dma_start(wq[:, 4:8, :], wqs[:, 4:8, :])
            nc.sync.dma_start(x0[:, 4:8, :], xs[:, 0, 4:8, :])
            nc.sync.dma_start(wk[:], wks[:])
            nc.sync.dma_start(wv[:], wvs[:])
            x1 = xtp.tile([128, 8, 512], BF16, tag="xt")
            nc.sync.dma_start(x1[:, 0:4, :], xs[:, 1, 0:4, :])
            nc.sync.dma_start(x1[:, 4:8, :], xs[:, 1, 4:8, :])
            nc.sync.dma_start(wo[:], wos[:])
            xtiles = [x0, x1]

            nc.vector.memset(onesb[:], 1.0)
            # trimask[r, hh, y] = 1 if y >= r else 0
            nc.vector.memset(trimask[:], 1.0)
            nc.gpsimd.affine_select(
                out=trimask[:],
                in_=trimask[:],
                compare_op=mybir.AluOpType.is_ge,
                fill=0.0,
                base=0,
                pattern=[[0, 2], [1, 128]],
                channel_multiplier=-1,
            )
            for j in range(1, 4):
                for hp in range(2):
                    nc.vector.memset(ptdiag[j][hp][:, :, 0 : 128 * j], 0.0)

            # ---- phase 1: projections for x chunk tci ----
            def phase1(tci):
                if tci < 2:
                    xt = xtiles[tci]
                else:
                    xt = xtp.tile([128, 8, 512], BF16, tag="xt")
                    nc.sync.dma_start(xt[:, 0:4, :], xs[:, tci, 0:4, :])
                    nc.sync.dma_start(xt[:, 4:8, :], xs[:, tci, 4:8, :])
                for w_sb, dst in ((wq, qt), (wk, kt)):
                    for ct in range(2):
                        ps = auxp.tile([128, 512], F32, tag="aux")
                        for di in range(8):
                            nc.tensor.matmul(
                                ps[:],
                                w_sb[:, di, ct * 128 : (ct + 1) * 128],
                                xt[:, di, :],
                                start=(di == 0),
                                stop=(di == 7),
                            )
                        nc.vector.tensor_copy(
                            dst[:, ct, tci * 512 : (tci + 1) * 512], ps[:]
                        )
                for tt in range(4):
                    ps = auxp.tile([128, 256], F32, tag="aux")
                    for di in range(8):
                        nc.tensor.matmul(
                            ps[:],
                            xt[:, di, tt * 128 : (tt + 1) * 128],
                            wv[:, di, :],
                            start=(di == 0),
                            stop=(di == 7),
                        )
                    kti = tci * 4 + tt
                    nc.vector.tensor_copy(
                        vp[:, kti, :, :],
                        ps[:].rearrange("p (h d) -> p h d", h=HPC),
                    )

            # ---- phase 2: attention for q chunk qc ----
            def phase2(qc):
                q0 = qc * 512
                n_kt = 4 * (qc + 1)
                opv = [
                    pvp.tile([128, 512], F32, tag=f"opv{hp}", name=f"opv{hp}")
                    for hp in range(2)
                ]
                sums = pvp.tile([128, 512], F32, tag="sums")

                def scores_exp(kti):
                    diag = kti >= 4 * qc
                    j = kti - 4 * qc if diag else 0
                    off = 128 * j
                    k0 = kti * 128
                    pts = []
                    for hp in range(2):
                        st = stp.tile([128, 2, 512], F32, tag="st")
                        for hh in range(2):
                            po = 64 * hh
                            for kh in range(2):
                                nc.tensor.matmul(
                                    st[64 * kh : 64 * kh + 64, hh, :],
                                    kt[
                                        po : po + 64,
                                        hp,
                                        k0 + 64 * kh : k0 + 64 * kh + 64,
                                    ],
                                    qt[po : po + 64, hp, q0 : q0 + 512],
                                    start=True,
                                    stop=True,
                                    tile_position=(po, 64 * kh),
                                )
                        if diag:
                            pt = ptdiag[j][hp]
                            nc.scalar.activation(
                                pt[:, :, off:], st[:, :, off:], EXP, scale=0.125
                            )
                            nc.gpsimd.tensor_mul(
                                pt[:, :, off : off + 128],
                                pt[:, :, off : off + 128],
                                trimask[:],
                            )
                        else:
                            pt = ptp.tile([128, 2, 512], BF16, tag="pt")
                            nc.scalar.activation(pt[:], st[:], EXP, scale=0.125)
                        pts.append(pt)
                    return pts

                def pv_sums(kti, pts, last):
                    first = kti == 0
                    for hp in range(2):
                        for hh in range(2):
                            nc.tensor.matmul(
                                opv[hp][64 * hh : 64 * hh + 64, :],
                                vp[:, kti, 2 * hp + hh, :],
                                pts[hp][:, hh, :],
                                start=first,
                                stop=last,
                                tile_position=(0, 64 * hh),
                            )
                    for q in range(4):
                        hp, hh = divmod(q, 2)
                        nc.tensor.matmul(
                            sums[32 * q : 32 * q + 1, :],
                            onesb[:, q : q + 1],
                            pts[hp][:, hh, :],
                            start=first,
                            stop=last,
                            tile_position=(0, 32 * q),
                        )

                prev = None
                for kti in range(n_kt):
                    cur = scores_exp(kti)
                    if prev is not None:
                        pv_sums(kti - 1, prev, last=False)
                    prev = cur
                pv_sums(n_kt - 1, prev, last=True)

                # normalize: recip of 4 sum rows, broadcast, multiply from PSUM
                ssb = normp.tile([128, 512], F32, tag="ssb")
                nc.vector.tensor_copy(ssb[:], sums[:])
                srec = normp.tile([16, 128], F32, tag="srec")
                for q in range(4):
                    nc.sync.dma_start(
                        srec[4 * q : 4 * q + 4, :], ssb[32 * q : 32 * q + 1, :]
                    )
                nc.vector.reciprocal(srec[:], srec[:])
                for q in range(4):
                    hp, hh = divmod(q, 2)
                    rrow = normp.tile([1, 512], F32, tag="rrow")
                    nc.sync.dma_start(rrow[:], srec[4 * q : 4 * q + 4, :])
                    rb = normp.tile([64, 512], F32, tag="rb")
                    nc.gpsimd.partition_broadcast(rb[:], rrow[:])
                    nc.vector.tensor_mul(
                        at[64 * hh : 64 * hh + 64, hp, q0 : q0 + 512],
                        opv[hp][64 * hh : 64 * hh + 64, :],
                        rb[:],
                    )

            # ---- phase 3: output projection for q chunk qc ----
            def phase3(qc):
                for tt in range(4):
                    qti = qc * 4 + tt
                    for nn in range(2):
                        po3 = auxp.tile([128, 512], F32, tag="aux")
                        for ctt in range(2):
                            nc.tensor.matmul(
                                po3[:],
                                at[:, ctt, qti * 128 : (qti + 1) * 128],
                                wo[:, ctt, nn * 512 : (nn + 1) * 512],
                                start=(ctt == 0),
                                stop=(ctt == 1),
                            )
                        ot = outp.tile([128, 512], F32, tag="ot")
                        nc.vector.tensor_copy(ot[:], po3[:])
                        nc.sync.dma_start(
                            out[
                                qti * 128 : (qti + 1) * 128,
                                nn * 512 : (nn + 1) * 512,
                            ],
                            ot[:],
                        )

            for tci in range(4):
                phase1(tci)
                phase2(tci)
                if tci >= 1:
                    phase3(tci - 1)
            phase3(3)
    nc.compile()
    return nc


def _get_nc():
    if "nc" not in _CACHE:
        _CACHE["nc"] = _build()
    return _CACHE["nc"]


def _in_maps(x, Wq, Wk, Wv, Wo):
    bf = ml_dtypes.bfloat16
    x = np.asarray(x, dtype=np.float32)
    Wq = np.asarray(Wq, dtype=np.float32)
    Wk = np.asarray(Wk, dtype=np.float32)
    Wv = np.asarray(Wv, dtype=np.float32)
    Wo = np.asarray(Wo, dtype=np.float32)
    maps = []
    for core in range(N_CORES):
        b, g = divmod(core, 4)
        sl = slice(g * GC, (g + 1) * GC)
        # xs[p, tc, dt, t] = x[b, tc*512+t, dt*128+p]
        xsw = np.ascontiguousarray(
            x[b].reshape(4, 512, 8, 128).transpose(3, 0, 2, 1)
        ).astype(bf)
        # w[p, dt, c] = W[sl][c, dt*128+p]
        wqw = np.ascontiguousarray(
            Wq[sl].reshape(GC, 8, 128).transpose(2, 1, 0)
        ).astype(bf)
        wkw = np.ascontiguousarray(
            Wk[sl].reshape(GC, 8, 128).transpose(2, 1, 0)
        ).astype(bf)
        wvw = np.ascontiguousarray(
            Wv[sl].reshape(GC, 8, 128).transpose(2, 1, 0)
        ).astype(bf)
        # wo[p, ct, n] = Wo[n, g*256 + ct*128 + p]
        wow = np.ascontiguousarray(
            Wo[:, sl].reshape(D, 2, 128).transpose(2, 1, 0)
        ).astype(bf)
        maps.append(
            {"xs": xsw, "wqs": wqw, "wks": wkw, "wvs": wvw, "wos": wow}
        )
    return maps


def _run(x, Wq, Wk, Wv, Wo, **spmd_kwargs):
    nc = _get_nc()
    res = run_bass_kernel_spmd(
        nc, _in_maps(x, Wq, Wk, Wv, Wo), core_ids=list(range(N_CORES)), **spmd_kwargs
    )
    outs = [r["out"] for r in res.results]
    full = np.stack(
        [
            outs[0] + outs[1] + outs[2] + outs[3],
            outs[4] + outs[5] + outs[6] + outs[7],
        ]
    ).astype(np.float32)
    return full, res


def kernel(x, Wq, Wk, Wv, Wo):
    full, _ = _run(x, Wq, Wk, Wv, Wo)
    return full


# revision 9
# speedup vs baseline: 1.0765x; 1.0765x over previous
"""Causal multi-head self-attention (B=2, T=2048, D=1024, H=16) on 8 TRN2
NeuronCores.

Sharding (Megatron-style, hardcoded): core = 4*b + g where b in {0,1} is the
batch and g in {0..3} a group of 4 heads. Each core computes Q/K/V projections
for its head group from x[b], fused causal attention for those 4 heads, and a
partial output projection against its 256-column slice of Wo. The host sums
the 4 partial outputs per batch (the all-reduce after out_proj).

v3 design (all matmul operands bf16, fp32 PSUM accumulation):
 - Scores per (kti, head-pair): a 4-way tiled quad of M=64 matmuls covering
   all 16 PE subarray quadrants (rows split by head, cols by k-half) -> both
   heads' score tiles land in one 512-cycle stream.
 - PV per (kti, head-pair): col-tiled M=64 pair (head0 -> PSUM rows 0:64,
   head1 -> 64:128) accumulating over kti.
 - Softmax denominators: 4-way col-tiled quad of M=1 ones-matmuls (one per
   head) accumulating into one PSUM bank at partitions {0,32,64,96}.
 - The PE instruction stream is hand-interleaved: the engine queues execute
   in order, so projection (next chunk) and output-projection (previous
   chunk) matmul groups are emitted INSIDE the attention kti loop, sized so
   the scalar engine's exp throughput (the attention pace-setter) is always
   covered by independent PE work:
     qc=0: Q/K/V projections of chunk 1      qc=1: projections of chunk 2
     qc=2: Q/K projections of chunk 3 + out-proj of chunk 0
     qc=3: V projections of chunk 3 (early, feeding the last k-tiles) +
           out-proj of chunks 1 and 2;  out-proj of chunk 3 trails.
 - Per-chunk tensors (qt/kt/vp/at split by chunk, at also by head-pair) so
   Tile's per-tensor dependency tracking cannot create false cross-phase
   serialization.
 - Causal diagonal tiles: exp sliced to the valid columns; the 128-wide
   triangle is masked via a precomputed lower-tri bf16 mask multiply on
   GpSimd; dead columns live in dedicated per-(offset, head-pair) pt tiles
   zeroed once at startup.
 - Normalize: evict denominators, gather rows {0,32,64,96} to [16,128], one
   multi-pass DVE reciprocal, scatter, gpsimd partition_broadcast, DVE
   multiply reading the PV accumulator directly from PSUM, writing bf16 at.
 - Input DMAs spread over two queues and chunked so the first projection
   matmul starts a few microseconds in.
"""

import numpy as np
import ml_dtypes

import concourse.bass as bass
import concourse.tile as tile
from concourse import bacc, mybir
from concourse.bass_utils import run_bass_kernel_spmd

B, T, D, H, DH = 2, 2048, 1024, 16, 64
HPC = 4  # heads per core
GC = 256  # projection columns per core (HPC * DH)
N_CORES = 8
F32 = mybir.dt.float32
BF16 = mybir.dt.bfloat16
EXP = mybir.ActivationFunctionType.Exp

_CACHE = {}


def _build():
    nc = bacc.Bacc(
        "TRN2", target_bir_lowering=False, debug=False, num_devices=N_CORES
    )
    # Pre-swizzled bf16 inputs (host does transposes + cast):
    #   xs[p, tc, dt, t] = x[b, tc*512+t, dt*128+p]
    #   wq/wk/wv[p, dt, c] = W[g*256+c, dt*128+p]
    #   wo[p, ct, n] = Wo[n, g*256 + ct*128 + p]
    xs = nc.dram_tensor("xs", [128, 4, 8, 512], BF16, kind="ExternalInput").ap()
    wqs = nc.dram_tensor("wqs", [128, 8, GC], BF16, kind="ExternalInput").ap()
    wks = nc.dram_tensor("wks", [128, 8, GC], BF16, kind="ExternalInput").ap()
    wvs = nc.dram_tensor("wvs", [128, 8, GC], BF16, kind="ExternalInput").ap()
    wos = nc.dram_tensor("wos", [128, 2, D], BF16, kind="ExternalInput").ap()
    out = nc.dram_tensor("out", [T, D], F32, kind="ExternalOutput").ap()

    with tile.TileContext(nc) as tc:
        with (
            tc.tile_pool(name="persist", bufs=1) as persist,
            tc.tile_pool(name="xtp", bufs=4) as xtp,
            tc.tile_pool(name="ptp", bufs=4) as ptp,
            tc.tile_pool(name="normp", bufs=2) as normp,
            tc.tile_pool(name="outp", bufs=2) as outp,
            # PSUM (8 banks): opv0+opv1+sums = 3, st-rotation (2 banks) x2 = 4
            # (shared by attention st tiles and phase-3 po3 tiles), aux = 1
            tc.tile_pool(name="pvp", bufs=1, space="PSUM") as pvp,
            tc.tile_pool(name="stp", bufs=2, space="PSUM") as stp,
            tc.tile_pool(name="auxp", bufs=1, space="PSUM") as auxp,
        ):
            wq = persist.tile([128, 8, GC], BF16, tag="wq")
            wk = persist.tile([128, 8, GC], BF16, tag="wk")
            wv = persist.tile([128, 8, GC], BF16, tag="wv")
            wo = persist.tile([128, 2, D], BF16, tag="wo")
            onesb = persist.tile([128, 4], BF16, tag="onesb")
            trimask = persist.tile([128, 2, 128], BF16, tag="trimask")
            # per-chunk projection outputs (separate tiles -> no false deps)
            qts = [
                persist.tile([128, 2, 512], BF16, tag=f"qt{t}", name=f"qt{t}")
                for t in range(4)
            ]
            kts = [
                persist.tile([128, 2, 512], BF16, tag=f"kt{t}", name=f"kt{t}")
                for t in range(4)
            ]
            vps = [
                persist.tile([128, 4, HPC, DH], BF16, tag=f"vp{t}", name=f"vp{t}")
                for t in range(4)
            ]
            # normalized attention output per (chunk, head-pair)
            ats = [
                [persist.tile([128, 512], BF16, tag=f"at{t}_{hp}",
                              name=f"at{t}_{hp}") for hp in range(2)]
                for t in range(4)
            ]
            # dedicated diagonal pt tiles per (offset j, head pair hp);
            # cols [0, 128j) are zeroed once and never rewritten
            ptdiag = [
                [persist.tile([128, 2, 512], BF16, tag=f"ptd{j}_{hp}",
                              name=f"ptd{j}_{hp}")
                 for hp in range(2)]
                for j in range(4)
            ]

            # Input DMAs: wq/wo on the scalar queue, the rest on sync, ordered
            # by first use. x tiles all prefetch up front (bufs=4).
            nc.scalar.dma_start(wq[:, 0:4, :], wqs[:, 0:4, :])
            xt_all = []
            for t in range(4):
                xti = xtp.tile([128, 8, 512], BF16, tag="xt", name=f"xt{t}")
                xt_all.append(xti)
            nc.sync.dma_start(xt_all[0][:, 0:4, :], xs[:, 0, 0:4, :])
            nc.scalar.dma_start(wq[:, 4:8, :], wqs[:, 4:8, :])
            nc.sync.dma_start(xt_all[0][:, 4:8, :], xs[:, 0, 4:8, :])
            nc.scalar.dma_start(wk[:], wks[:])
            nc.scalar.dma_start(wv[:], wvs[:])
            nc.sync.dma_start(xt_all[1][:], xs[:, 1])
            nc.sync.dma_start(xt_all[2][:], xs[:, 2])
            nc.sync.dma_start(xt_all[3][:], xs[:, 3])
            nc.scalar.dma_start(wo[:], wos[:])

            nc.vector.memset(onesb[:], 1.0)
            # trimask[r, hh, y] = 1 if y >= r else 0
            nc.vector.memset(trimask[:], 1.0)
            nc.gpsimd.affine_select(
                out=trimask[:],
                in_=trimask[:],
                compare_op=mybir.AluOpType.is_ge,
                fill=0.0,
                base=0,
                pattern=[[0, 2], [1, 128]],
                channel_multiplier=-1,
            )
            for j in range(1, 4):
                for hp in range(2):
                    nc.vector.memset(ptdiag[j][hp][:, :, 0 : 128 * j], 0.0)

            # ---- phase 1 groups: projections for x chunk tci ----
            def phase1_groups(tci):
                xt = xt_all[tci]

                def qk_group(w_sb, dst, ct):
                    def emit():
                        ps = auxp.tile([128, 512], F32, tag="aux", name="ps_qk")
                        for di in range(8):
                            nc.tensor.matmul(
                                ps[:],
                                w_sb[:, di, ct * 128 : (ct + 1) * 128],
                                xt[:, di, :],
                                start=(di == 0),
                                stop=(di == 7),
                            )
                        nc.vector.tensor_copy(dst[:, ct, :], ps[:])
                    return emit

                def v_group(tt):
                    def emit():
                        ps = auxp.tile([128, 256], F32, tag="aux", name="ps_v")
                        for di in range(8):
                            nc.tensor.matmul(
                                ps[:],
                                xt[:, di, tt * 128 : (tt + 1) * 128],
                                wv[:, di, :],
                                start=(di == 0),
                                stop=(di == 7),
                            )
                        nc.vector.tensor_copy(
                            vps[tci][:, tt, :, :],
                            ps[:].rearrange("p (h d) -> p h d", h=HPC),
                        )
                    return emit

                qk = [
                    qk_group(w_sb, dst, ct)
                    for w_sb, dst in ((wq, qts[tci]), (wk, kts[tci]))
                    for ct in range(2)
                ]
                vg = [v_group(tt) for tt in range(4)]
                return qk, vg

            # ---- phase 3 groups: output projection for q chunk qc ----
            def phase3_groups(qc):
                def o_group(tt):
                    def emit():
                        qti = qc * 4 + tt
                        po3 = stp.tile([128, 2, 512], F32, tag="st", name="po3")
                        for nn in range(2):
                            for ctt in range(2):
                                nc.tensor.matmul(
                                    po3[:, nn, :],
                                    ats[qc][ctt][:, tt * 128 : (tt + 1) * 128],
                                    wo[:, ctt, nn * 512 : (nn + 1) * 512],
                                    start=(ctt == 0),
                                    stop=(ctt == 1),
                                )
                        ot = outp.tile([128, 2, 512], F32, tag="ot")
                        nc.vector.tensor_copy(ot[:], po3[:])
                        nc.sync.dma_start(
                            out[qti * 128 : (qti + 1) * 128, :].rearrange(
                                "q (a n) -> q a n", a=2
                            ),
                            ot[:],
                        )
                    return emit

                return [o_group(tt) for tt in range(4)]

            # ---- phase 2: attention for q chunk qc, interleaving fills ----
            def phase2(qc, fills):
                q0 = qc * 512
                n_kt = 4 * (qc + 1)
                opv = [
                    pvp.tile([128, 512], F32, tag=f"opv{hp}", name=f"opv{hp}")
                    for hp in range(2)
                ]
                sums = pvp.tile([128, 512], F32, tag="sums", name="sums")

                def scores_exp(kti):
                    diag = kti >= 4 * qc
                    j = kti - 4 * qc if diag else 0
                    off = 128 * j
                    tci, tk = divmod(kti, 4)
                    k0 = tk * 128
                    pts = []
                    for hp in range(2):
                        st = stp.tile([128, 2, 512], F32, tag="st", name="st")
                        for hh in range(2):
                            po = 64 * hh
                            for kh in range(2):
                                nc.tensor.matmul(
                                    st[64 * kh : 64 * kh + 64, hh, :],
                                    kts[tci][
                                        po : po + 64,
                                        hp,
                                        k0 + 64 * kh : k0 + 64 * kh + 64,
                                    ],
                                    qts[qc][po : po + 64, hp, :],
                                    start=True,
                                    stop=True,
                                    tile_position=(po, 64 * kh),
                                )
                        if diag:
                            pt = ptdiag[j][hp]
                            nc.scalar.activation(
                                pt[:, :, off:], st[:, :, off:], EXP, scale=0.125
                            )
                            nc.gpsimd.tensor_mul(
                                pt[:, :, off : off + 128],
                                pt[:, :, off : off + 128],
                                trimask[:],
                            )
                        else:
                            pt = ptp.tile([128, 2, 512], BF16, tag="pt")
                            nc.scalar.activation(pt[:], st[:], EXP, scale=0.125)
                        pts.append(pt)
                    return pts

                def pv_sums(kti, pts, last):
                    first = kti == 0
                    tci, tk = divmod(kti, 4)
                    for hp in range(2):
                        for hh in range(2):
                            nc.tensor.matmul(
                                opv[hp][64 * hh : 64 * hh + 64, :],
                                vps[tci][:, tk, 2 * hp + hh, :],
                                pts[hp][:, hh, :],
                                start=first,
                                stop=last,
                                tile_position=(0, 64 * hh),
                            )
                    for q in range(4):
                        hp, hh = divmod(q, 2)
                        nc.tensor.matmul(
                            sums[32 * q : 32 * q + 1, :],
                            onesb[:, q : q + 1],
                            pts[hp][:, hh, :],
                            start=first,
                            stop=last,
                            tile_position=(0, 32 * q),
                        )

                emitted = 0
                prev = None
                for kti in range(n_kt):
                    cur = scores_exp(kti)
                    if prev is not None:
                        pv_sums(kti - 1, prev, last=False)
                    prev = cur
                    want = (kti + 1) * len(fills) // n_kt
                    while emitted < want:
                        fills[emitted]()
                        emitted += 1
                pv_sums(n_kt - 1, prev, last=True)
                while emitted < len(fills):
                    fills[emitted]()
                    emitted += 1

                # normalize: recip of 4 denominator rows, broadcast, multiply
                ssb = normp.tile([128, 512], F32, tag="ssb")
                nc.vector.tensor_copy(ssb[:], sums[:])
                srec = normp.tile([16, 128], F32, tag="srec")
                for q in range(4):
                    nc.sync.dma_start(
                        srec[4 * q : 4 * q + 4, :], ssb[32 * q : 32 * q + 1, :]
                    )
                nc.vector.reciprocal(srec[:], srec[:])
                for q in range(4):
                    hp, hh = divmod(q, 2)
                    rrow = normp.tile([1, 512], F32, tag="rrow")
                    nc.sync.dma_start(rrow[:], srec[4 * q : 4 * q + 4, :])
                    rb = normp.tile([64, 512], F32, tag="rb")
                    nc.gpsimd.partition_broadcast(rb[:], rrow[:])
                    nc.vector.tensor_mul(
                        ats[qc][hp][64 * hh : 64 * hh + 64, :],
                        opv[hp][64 * hh : 64 * hh + 64, :],
                        rb[:],
                    )

            # ---- the hand-interleaved schedule ----
            qk1, vg1 = phase1_groups(0)
            for g in qk1 + vg1:
                g()
            p1_qk, p1_v = {}, {}
            for t in (1, 2, 3):
                p1_qk[t], p1_v[t] = phase1_groups(t)
            phase2(0, p1_qk[1] + p1_v[1])
            phase2(1, p1_qk[2] + p1_v[2])
            phase2(2, p1_qk[3] + phase3_groups(0))
            phase2(3, p1_v[3] + phase3_groups(1) + phase3_groups(2))
            for g in phase3_groups(3):
                g()
    nc.compile()
    return nc


def _get_nc():
    if "nc" not in _CACHE:
        _CACHE["nc"] = _build()
    return _CACHE["nc"]


def _in_maps(x, Wq, Wk, Wv, Wo):
    bf = ml_dtypes.bfloat16
    x = np.asarray(x, dtype=np.float32)
    Wq = np.asarray(Wq, dtype=np.float32)
    Wk = np.asarray(Wk, dtype=np.float32)
    Wv = np.asarray(Wv, dtype=np.float32)
    Wo = np.asarray(Wo, dtype=np.float32)
    maps = []
    for core in range(N_CORES):
        b, g = divmod(core, 4)
        sl = slice(g * GC, (g + 1) * GC)
        # xs[p, tc, dt, t] = x[b, tc*512+t, dt*128+p]
        xsw = np.ascontiguousarray(
            x[b].reshape(4, 512, 8, 128).transpose(3, 0, 2, 1)
        ).astype(bf)
        # w[p, dt, c] = W[sl][c, dt*128+p]
        wqw = np.ascontiguousarray(
            Wq[sl].reshape(GC, 8, 128).transpose(2, 1, 0)
        ).astype(bf)
        wkw = np.ascontiguousarray(
            Wk[sl].reshape(GC, 8, 128).transpose(2, 1, 0)
        ).astype(bf)
        wvw = np.ascontiguousarray(
            Wv[sl].reshape(GC, 8, 128).transpose(2, 1, 0)
        ).astype(bf)
        # wo[p, ct, n] = Wo[n, g*256 + ct*128 + p]
        wow = np.ascontiguousarray(
            Wo[:, sl].reshape(D, 2, 128).transpose(2, 1, 0)
        ).astype(bf)
        maps.append(
            {"xs": xsw, "wqs": wqw, "wks": wkw, "wvs": wvw, "wos": wow}
        )
    return maps


def _run(x, Wq, Wk, Wv, Wo, **spmd_kwargs):
    nc = _get_nc()
    res = run_bass_kernel_spmd(
        nc, _in_maps(x, Wq, Wk, Wv, Wo), core_ids=list(range(N_CORES)), **spmd_kwargs
    )
    outs = [r["out"] for r in res.results]
    full = np.stack(
        [
            outs[0] + outs[1] + outs[2] + outs[3],
            outs[4] + outs[5] + outs[6] + outs[7],
        ]
    ).astype(np.float32)
    return full, res


def kernel(x, Wq, Wk, Wv, Wo):
    full, _ = _run(x, Wq, Wk, Wv, Wo)
    return full
